# revision 50
# baseline (speedup 1.0000x reference)
"""Causal multi-head attention (ChunkedDotProdAttention) on 8 TRN2 NeuronCores.

Problem: q,k,v [2, 2048, 2048] f32, 16 heads of dh=128, causal mask
(masked scores set to -50000 -> softmax -> exactly 0 in f32), out = attn @ v.

Sharding: 32 (batch, head) pairs, 4 per core; each core computes full
attention for its pairs — no cross-device comm.

Per-core kernel layout (everything transposed):
  - host pre-transposes q,k to [dh, n] (bf16); v host-packed for flat DMA
  - S^T[k, q] = K_j^T.T @ Q^T per (key-block j, 1024-query chunk c), causal
    blocks only, narrowed to valid queries
  - P^T = exp(scale * S^T): split across two engines — most blocks on ACT
    (exact exp, 1 elem/lane/cycle is the ACT floor), a tuned ~40% of
    columns on DVE via a Schraudolph bit-trick (i16 = S*A + B; the int16
    bit pattern IS bf16 exp(scale*S) to ~1.8% rms, end-to-end rel err
    1.35e-2 < 2e-2 gate). ACT alone would be the bottleneck at ~76us/core.
  - P^T tiles packed contiguously per chunk (base offsets, no junk columns)
  - diagonal blocks: triangular zero via gpsimd affine_select (Pool)
  - out^T[d, q] += V_j.T @ P^T_j accumulated in PSUM (V_j natural layout);
    psum->sbuf out^T drain on ACT, split per psum bank so half overlaps
    the causal ramp (gpsimd cannot read PSUM on hw)
  - softmax denominators: TWO parallel bf16 accumulator chains per chunk
    (DVE tensor_tensor 2x mode + otherwise-idle Pool gpsimd), exported raw
    to DRAM; host does the 128-partition sum + division
PE work = QK + PV streaming = 58us/core, the roofline for bf16 causal.
"""

import numpy as np
import ml_dtypes

B = 2
N = 2048
D_MODEL = 2048
H = 16
DH = 128
N_CORES = 8
PAIRS_PER_CORE = (B * H) // N_CORES  # 4
SCALE = float(DH) ** -0.5
CHUNK = 1024  # query chunk (2 psum banks; one key-block row per S tile)
NCHUNKS = N // CHUNK  # 2
QB = 128  # query/key block
NB = N // QB  # 16 key blocks
BLOCKS_PER_CHUNK = CHUNK // QB  # 8
MMN = 512  # max matmul free dim (one psum bank of f32)

# --- packed P^T layout ------------------------------------------------------
def _qoff(c, j):
    return max(0, j * QB - c * CHUNK)


_BASES = []  # per chunk: list of col bases in the packed pt tile
_PTW = []  # per chunk: total packed width
for _c in range(NCHUNKS):
    _jc = BLOCKS_PER_CHUNK * (_c + 1)
    _bases = []
    _w = 0
    for _j in range(_jc):
        _bases.append(_w)
        _w += CHUNK - _qoff(_c, _j)
    _BASES.append(_bases)
    _PTW.append(_w)

# --- engine assignment ------------------------------------------------------
# exp: ACT (exact) or DVE (Schraudolph bit trick, ~1.8% rms per element).
# GPSIMD cannot touch PSUM on hw, so Pool can't exp. Sheds ~20us/core of
# ACT time; end-to-end rel err ~9e-3 (< 2e-2 gate).
DVE_EXP = {(0, 0), (0, 2), (0, 4), (1, 1), (1, 3), (1, 5), (1, 7), (1, 9)}
# denominator adds: two parallel accumulator chains per chunk — acc0 fed by
# DVE (tensor_tensor, 2x mode), acc1 fed by Pool (gpsimd add: ~2ns/col on
# hw, usable only because Pool is otherwise idle). Sets name the
# Pool-chain blocks. Chain INITS always run on DVE (pool copy is 3.5us!).
POOL_ADD = {(0, 1), (0, 3), (0, 5), (0, 7), (1, 1), (1, 3), (1, 5), (1, 7)}
# static-schedule priority offsets (higher = hoisted earlier)
EXP_PRIO_EARLY = 45
EXP_PRIO = 14
ADD_PRIO = -30
QK_PRIO_EARLY = 40
QK_PRIO = 12
# software pipelining: interleave the last R blocks of each chunk's causal
# ramp 1:1 with the next chunk's first blocks (PE fills ramp bubbles with
# the next chunk's QKs). INTRA: chunk0->chunk1 within a pair; INTER: a
# pair's chunk1 ramp with the next pair's chunk0.
R_INTRA = 0
R_INTER = 0
PS_S_BUFS = 2
PS_O_BUFS = 2

# Schraudolph constants: bf16 bits of exp(SCALE*s) ~= s*SCH_A + SCH_B
SCH_C = 5.5
SCH_A = (2.0**7 / float(np.log(2.0))) * SCALE
SCH_B = 127.0 * 2.0**7 - SCH_C + 0.5  # +0.5: float->int converts truncate

# first block of each accumulator chain per chunk (chain 0 = DVE, 1 = Pool):
# acc cols before that block's qoff are never written -> host zeroes them
_CHAIN_START = []
for _c in range(NCHUNKS):
    _jc = BLOCKS_PER_CHUNK * (_c + 1)
    _dve_first = min(_j for _j in range(_jc) if (_c, _j) not in POOL_ADD)
    _pool_js = [_j for _j in range(_jc) if (_c, _j) in POOL_ADD]
    _pool_first = min(_pool_js) if _pool_js else None
    _CHAIN_START.append(
        (_qoff(_c, _dve_first), None if _pool_first is None else _qoff(_c, _pool_first))
    )

_nc_cache = {}
_last_in_maps = None


def _build_nc(reps=0):
    """Build + compile the per-core Bass kernel (same NEFF for all cores).

    reps>0 wraps the body in a dynamic For_i loop running it `reps` times —
    used only for wall-clock benchmarking (the work is idempotent)."""
    from contextlib import ExitStack

    import concourse.bass as bass
    import concourse.mybir as mybir
    import concourse.tile as tile
    from concourse import bacc
    from concourse import bass_isa

    dt_mm = mybir.dt.bfloat16
    f32 = mybir.dt.float32
    i16 = mybir.dt.int16

    nc = bacc.Bacc(
        "TRN2",
        target_bir_lowering=False,
        debug=False,
        enable_asserts=False,
        num_devices=N_CORES,
    )
    P = PAIRS_PER_CORE
    qT_d = nc.dram_tensor("qT", [P, DH, N], dt_mm, kind="ExternalInput").ap()
    kT_d = nc.dram_tensor("kT", [P, DH, N], dt_mm, kind="ExternalInput").ap()
    # v pre-arranged on host to the SBUF layout: [pair, k_local, block*DH+d]
    v_d = nc.dram_tensor("v", [P, QB, NB * DH], dt_mm, kind="ExternalInput").ap()
    outT_d = nc.dram_tensor("outT", [P, DH, N], f32, kind="ExternalOutput").ap()
    # raw softmax-denominator accumulators (2 parallel chains per chunk);
    # the host does the partition-sum and the division
    accs_d = nc.dram_tensor(
        "accs", [P, NCHUNKS, 2, QB, CHUNK], dt_mm, kind="ExternalOutput"
    ).ap()

    with tile.TileContext(nc) as tc, ExitStack() as ctx:
        sb = ctx.enter_context(tc.tile_pool(name="sb", bufs=3))
        pt_pool = ctx.enter_context(tc.tile_pool(name="pt", bufs=2))
        acc_pool = ctx.enter_context(tc.tile_pool(name="acc", bufs=4))
        outp = ctx.enter_context(tc.tile_pool(name="outp", bufs=2))
        # S-slot count deepens the QK->exp->PV pipeline to hide exp latency;
        # 3+1 trades o double-buffering for a deeper QK queue (8 banks total)
        ps_s = ctx.enter_context(
            tc.tile_pool(name="ps_s", bufs=PS_S_BUFS, space="PSUM")
        )
        ps_o = ctx.enter_context(
            tc.tile_pool(name="ps_o", bufs=PS_O_BUFS, space="PSUM")
        )

        # cache the affine_select fill constant in a Pool register once —
        # otherwise every affine emits its own reg_mov on Pool
        fill_reg = nc.gpsimd.to_reg(0.0)

        rep_ctx = ExitStack()
        if reps:
            # hint_engines: body >256 instrs/engine -> back-edge would
            # IRAM-miss (~4us) without prefetch hints
            rep_ctx.enter_context(
                tc.For_i(
                    0,
                    reps,
                    1,
                    hint_engines=(
                        mybir.EngineType.PE,
                        mybir.EngineType.Activation,
                        mybir.EngineType.DVE,
                        mybir.EngineType.Pool,
                        mybir.EngineType.SP,
                    ),
                )
            )

        def emit_block(st, c, j, jc):
            qoff = _qoff(c, j)
            base = _BASES[c][j]
            width = CHUNK - qoff
            # split [qoff, CHUNK) into <=MMN psum-bank-aligned pieces
            pieces = [(qoff, MMN), (MMN, CHUNK)] if qoff < MMN else [(qoff, CHUNK)]
            s_ps = ps_s.tile([128, CHUNK], f32, tag="s")
            # hoist QKs ahead of the previous block's exp-gated PV cluster
            # (and the previous chunk/pair tail) in the static schedule
            with tc.high_priority(offset=QK_PRIO_EARLY if j < 2 else QK_PRIO):
                for p0, p1 in pieces:
                    nc.tensor.matmul(
                        s_ps[:, p0:p1],
                        lhsT=st["kT"][:, j * QB : (j + 1) * QB],
                        rhs=st["qT"][:, c * CHUNK + p0 : c * CHUNK + p1],
                        start=True,
                        stop=True,
                    )
            pt = st["pt"]
            dst = pt[:, base : base + width]
            # hoist exps ahead of same-engine denominator adds (adds have
            # slack; exps gate the PV matmuls and the next QK's psum slot)
            with tc.high_priority(offset=EXP_PRIO_EARLY if j < 2 else EXP_PRIO):
                if (c, j) in DVE_EXP:
                    nc.vector.tensor_scalar(
                        dst.bitcast(i16),
                        s_ps[:, qoff:],
                        SCH_A,
                        SCH_B,
                        mybir.AluOpType.mult,
                        mybir.AluOpType.add,
                    )
                else:
                    nc.scalar.activation(
                        dst,
                        s_ps[:, qoff:],
                        mybir.ActivationFunctionType.Exp,
                        scale=SCALE,
                    )
            if j * QB >= c * CHUNK:  # diagonal block
                # strict-upper triangle of the first 128 valid cols:
                # keep where local_q - local_k >= 0
                nc.gpsimd.affine_select(
                    out=pt[:, base : base + QB],
                    in_=pt[:, base : base + QB],
                    compare_op=mybir.AluOpType.is_ge,
                    fill=fill_reg,
                    base=0,
                    channel_multiplier=-1,
                    pattern=[[1, QB]],
                )
            for p0, p1 in pieces:
                # stop on the LAST block writing each psum bank (bank0
                # [0,512) is last touched by j=3+8c; bank1 by jc-1) so the
                # sim's accumulation-group tracking sees every bank closed
                # before the out^T drain (stop is sim-only metadata on hw)
                last_j = (jc - 1) if p1 > MMN else (3 + 8 * c)
                nc.tensor.matmul(
                    st["o"][:, p0:p1],
                    lhsT=st["v"][:, j * DH : (j + 1) * DH],
                    rhs=pt[:, base + (p0 - qoff) : base + (p1 - qoff)],
                    start=(j == 0),
                    stop=(j == last_j),
                )
            if j == 3 + 8 * c:
                # bank0 of o is final after this block — drain its half now
                # (overlaps the causal ramp; the chunk-boundary tail then
                # only waits on the second half)
                nc.scalar.copy(
                    st["outT"][:, c * CHUNK : c * CHUNK + MMN], st["o"][:, :MMN]
                )
            # running denominator accumulation: two parallel chains (DVE
            # fast-path + otherwise-idle Pool) so neither engine serializes
            # the whole chunk
            pool_chain = (c, j) in POOL_ADD
            eng = nc.gpsimd if pool_chain else nc.vector
            acc = st["acc1"] if pool_chain else st["acc0"]
            fkey = "first1" if pool_chain else "first0"
            with tc.high_priority(offset=ADD_PRIO):
                if st[fkey]:
                    st[fkey] = False
                    # init always on DVE: 4x-mode copy (~330ns) vs 3.5us gpsimd
                    nc.vector.tensor_copy(acc[:, qoff:], dst)
                else:
                    eng.tensor_tensor(
                        acc[:, qoff:], acc[:, qoff:], dst, mybir.AluOpType.add
                    )

        def emit_tail(st, c):
            # out^T psum drain on ACT (gpsimd can't read PSUM); accs straight
            # out to DRAM — the host finishes the softmax division. outT
            # DMA'd per chunk so the final pair's tail is one [128,1024]
            nc.scalar.copy(
                st["outT"][:, c * CHUNK + MMN : (c + 1) * CHUNK], st["o"][:, MMN:]
            )
            nc.sync.dma_start(accs_d[st["p"], c, 0], st["acc0"][:])
            nc.sync.dma_start(accs_d[st["p"], c, 1], st["acc1"][:])
            nc.sync.dma_start(
                outT_d[st["p"]][:, c * CHUNK : (c + 1) * CHUNK],
                st["outT"][:, c * CHUNK : (c + 1) * CHUNK],
            )

        sts = {}

        def job_start(p):
            # DMA order = first-use order: the first QK needs qT[:MMN] +
            # kT[:QB], the first PVs need v[:4DH]; the rest follows
            qT_s = sb.tile([128, N], dt_mm, tag="qT")
            kT_s = sb.tile([128, N], dt_mm, tag="kT")
            v_s = sb.tile([128, NB * DH], dt_mm, tag="v")
            nc.sync.dma_start(qT_s[:, :MMN], qT_d[p][:, :MMN])
            nc.sync.dma_start(kT_s[:, :QB], kT_d[p][:, :QB])
            nc.sync.dma_start(qT_s[:, MMN:CHUNK], qT_d[p][:, MMN:CHUNK])
            nc.sync.dma_start(kT_s[:, QB : 4 * QB], kT_d[p][:, QB : 4 * QB])
            nc.sync.dma_start(v_s[:, : 4 * DH], v_d[p][:, : 4 * DH])
            nc.sync.dma_start(kT_s[:, 4 * QB :], kT_d[p][:, 4 * QB :])
            nc.sync.dma_start(v_s[:, 4 * DH :], v_d[p][:, 4 * DH :])
            nc.sync.dma_start(qT_s[:, CHUNK:], qT_d[p][:, CHUNK:])
            outT_s = outp.tile([128, N], f32, tag="outT")
            sts[p] = {"p": p, "qT": qT_s, "kT": kT_s, "v": v_s, "outT": outT_s}

        def job_cstart(p, c):
            st = sts[p]
            st["pt"] = pt_pool.tile([128, _PTW[c]], dt_mm, tag=f"pt{c}", name="pt")
            st["acc0"] = acc_pool.tile([128, CHUNK], dt_mm, tag="acc0", name="acc0")
            st["acc1"] = acc_pool.tile([128, CHUNK], dt_mm, tag="acc1", name="acc1")
            st["first0"] = True
            st["first1"] = True
            st["o"] = ps_o.tile([128, CHUNK], f32, tag="o", name="o")

        # software-pipelined emission: streams = (pair, chunk) in order; the
        # last R blocks of each stream interleave 1:1 with the next stream's
        # prologue + first R blocks, so PE fills causal-ramp bubbles with the
        # next chunk's QKs
        streams = []
        for p in range(P):
            for c in range(NCHUNKS):
                jc = BLOCKS_PER_CHUNK * (c + 1)
                pro = ([("start", p)] if c == 0 else []) + [("cstart", p, c)]
                blocks = [("blk", p, c, j, jc) for j in range(jc)]
                streams.append((pro, blocks, ("tail", p, c), c))

        seq = []
        emitted = 0  # blocks of the CURRENT stream already emitted by prior zone
        pro_done = [False] * len(streams)
        for k, (pro, blocks, tail, c) in enumerate(streams):
            if not pro_done[k]:
                seq.extend(pro)
                pro_done[k] = True
            n = len(blocks)
            if k + 1 < len(streams):
                nxt_pro, nxt_blocks, _, nxt_c = streams[k + 1]
                R = R_INTRA if nxt_c == 1 else R_INTER
                R = min(R, n - emitted, len(nxt_blocks))
            else:
                R = 0
            for j in range(emitted, n - R):
                seq.append(blocks[j])
            if R > 0:
                nxt_pro, nxt_blocks, _, _ = streams[k + 1]
                seq.extend(nxt_pro)
                pro_done[k + 1] = True
                for i in range(R):
                    seq.append(blocks[n - R + i])
                    seq.append(nxt_blocks[i])
            seq.append(tail)
            emitted = R

        for ev in seq:
            if ev[0] == "start":
                job_start(ev[1])
            elif ev[0] == "cstart":
                job_cstart(ev[1], ev[2])
            elif ev[0] == "blk":
                _, p, c, j, jc = ev
                emit_block(sts[p], c, j, jc)
            else:
                _, p, c = ev
                emit_tail(sts[p], c)

        rep_ctx.close()

    nc.compile()
    return nc


def _get_nc():
    if "nc" not in _nc_cache:
        _nc_cache["nc"] = _build_nc()
    return _nc_cache["nc"]


def kernel(q, k, v):
    from concourse.bass_utils import run_bass_kernel_spmd

    q = np.asarray(q, dtype=np.float32)
    k = np.asarray(k, dtype=np.float32)
    v = np.asarray(v, dtype=np.float32)

    bf16 = ml_dtypes.bfloat16
    # [b, n, h, dh] -> [b, h, dh, n] for q/k; v -> [b, h, k_local, block, dh]
    qT = np.ascontiguousarray(
        q.reshape(B, N, H, DH).transpose(0, 2, 3, 1)
    ).astype(bf16)
    kT = np.ascontiguousarray(
        k.reshape(B, N, H, DH).transpose(0, 2, 3, 1)
    ).astype(bf16)
    vh = np.ascontiguousarray(
        v.reshape(B, NB, QB, H, DH).transpose(0, 3, 2, 1, 4)
    ).astype(bf16)

    qT = qT.reshape(B * H, DH, N)
    kT = kT.reshape(B * H, DH, N)
    vh = vh.reshape(B * H, QB, NB * DH)

    in_maps = []
    for core in range(N_CORES):
        lo = core * PAIRS_PER_CORE
        hi = lo + PAIRS_PER_CORE
        in_maps.append(
            {
                "qT": np.ascontiguousarray(qT[lo:hi]),
                "kT": np.ascontiguousarray(kT[lo:hi]),
                "v": np.ascontiguousarray(vh[lo:hi]),
            }
        )

    global _last_in_maps
    _last_in_maps = in_maps

    nc = _get_nc()
    res = run_bass_kernel_spmd(nc, in_maps, core_ids=list(range(N_CORES)))

    # reassemble: outT per core [P, dh, n] f32 (unnormalized) -> normalize by
    # the softmax denominators (sum the raw accumulators over partitions),
    # then back to [b, n, h*dh]
    outT = np.concatenate([r["outT"] for r in res.results], axis=0)  # [32, dh, n]
    accs = np.concatenate([r["accs"] for r in res.results], axis=0)
    # accs: [32, NCHUNKS, 2, 128, CHUNK] bf16; zero never-written chain
    # prefixes, then partition-sum -> [32, n]
    accs = accs.astype(np.float32)
    for c in range(NCHUNKS):
        s0, s1 = _CHAIN_START[c]
        if s0:
            accs[:, c, 0, :, :s0] = 0.0
        if s1 is None:
            accs[:, c, 1] = 0.0
        elif s1:
            accs[:, c, 1, :, :s1] = 0.0
    sums = accs.sum(axis=(2, 3)).reshape(B * H, N)
    outT = outT / sums[:, None, :]
    out = outT.reshape(B, H, DH, N).transpose(0, 3, 1, 2).reshape(B, N, D_MODEL)
    return np.ascontiguousarray(out)
